# revision 18
# baseline (speedup 1.0000x reference)
"""HPSS (harmonic/percussive source separation) Trainium2 kernel, v5.

Input S [2,2,1025,1024] f32. Per (b,c) plane: harm = median-31 along W
(zero-padded), perc = median-31 along H; softmask with power=2, margin=1;
returns (S*mask_h, S*mask_p).

Sharding: 8 cores = 4 planes x 2 W-halves. Each core computes perc medians
for its 512 columns over rows 0..1024 and harm medians + softmask outputs
for rows 0..1023 x its 512 columns. Row 1024 is finished on the host.

Exact median-31 via Gil-Werman prefix/suffix order statistics in bf16.
Each level l only ever feeds 16 slots per 31-block downstream, so levels
live in compact [*, nb, 16] tiles; every level's x-window is pre-gathered
on the host into contiguous DRAM streams (XPL/XHL) so all DVE tensor ops
run on contiguous bf16 operands (2x mode). The layer merge accumulates
min-over-layers into parity-split tiles (even layers -> cminA at slot o,
odd -> cminB at o-1; both 4B-aligned) over a contiguous block range;
cross-strip/cross-q garbage blocks land in output slots nothing reads.
Perc medians stay in SBUF; the PE transposes them for the softmask.

v5: one mega-chunk per orientation (perc: 4 column strips stacked on the
free axis, harm: all 8 row groups) to amortize the ~180ns/scan fixed cost
and DVE dispatch gaps. Fits in SBUF by running the suffix chain first and
interleaving merge layer 15-l right after prefix level l, so only 3
rotating pre tiles are live. reciprocal_approx_fast for the softmask.
"""
import sys

import numpy as np

sys.path.insert(0, "/opt/trn_rl_repo")

P = 128
K = 31
KS = 32
LEV = 16
GUARD = 2.0
NB_P = 35          # perc blocks per strip (covers padded-H 1085)
NB_H = 18          # harm blocks per row group (covers 558-col strip)
NSTR = 4           # perc strips (all 512 cols in one chunk)
NQ = 8             # harm row groups (all 1024 rows in one chunk)
HALF = 15
NLI = 2 * LEV      # 32 level streams

_PROGRAM = None


def _build_program():
    from contextlib import ExitStack

    import concourse.mybir as mybir
    import concourse.tile as tile
    from concourse import bacc

    f32 = mybir.dt.float32
    bf16 = mybir.dt.bfloat16
    MIN = mybir.AluOpType.min
    MAX = mybir.AluOpType.max
    ADD = mybir.AluOpType.add
    MULT = mybir.AluOpType.mult
    SUB = mybir.AluOpType.subtract

    from bass_rust import ActivationFunctionType as AF

    nc = bacc.Bacc("TRN2", target_bir_lowering=False, debug=True)
    XPL = nc.declare_dram_parameter("XPL", [512, NLI * NB_P * 16], bf16,
                                    isOutput=False)
    XHL = nc.declare_dram_parameter("XHL", [1024, NLI * NB_H * 16], bf16,
                                    isOutput=False)
    XS = nc.declare_dram_parameter("XS", [1024, 512], bf16, isOutput=False)
    ID = nc.declare_dram_parameter("ID", [P, P], bf16, isOutput=False)
    GI = nc.declare_dram_parameter("GI", [P, (NQ * NB_H + 1) * KS], bf16,
                                   isOutput=False)
    OH = nc.declare_dram_parameter("OH", [1024, 512], bf16, isOutput=True)
    OP = nc.declare_dram_parameter("OP", [1024, 512], bf16, isOutput=True)
    PMR = nc.declare_dram_parameter("PMR", [512, 2], bf16, isOutput=True)

    WMAX = NQ * NB_H * 16  # 2304 compact slots (harm); perc uses 2240

    with tile.TileContext(nc) as tc:
        with ExitStack() as ctx:
            cpool = ctx.enter_context(tc.tile_pool(name="const", bufs=1))
            inpool = ctx.enter_context(tc.tile_pool(name="in", bufs=3))
            pool = ctx.enter_context(tc.tile_pool(name="work", bufs=1))
            spool = ctx.enter_context(tc.tile_pool(name="soft", bufs=1))
            ppool = ctx.enter_context(tc.tile_pool(name="ps", bufs=2,
                                                   space="PSUM"))

            mask = cpool.tile([P, WMAX], bf16)
            nc.vector.memset(mask[:], 0.0)
            nc.vector.memset(
                mask[:].rearrange("p (b k) -> p b k", k=16)[:, :, 0:1], 1e30)
            ident = cpool.tile([P, P], bf16)
            nc.sync.dma_start(ident[:], ID[:])

            suf = [pool.tile([P, WMAX], bf16, tag=f"suf{l}", name=f"suf{l}")
                   for l in range(LEV)]
            prer = [pool.tile([P, WMAX], bf16, tag=f"prer{i}", name=f"prer{i}")
                    for i in range(3)]

            def stream(dram, nb, ngrp, li):
                W = ngrp * nb * 16
                xt = inpool.tile([P, WMAX], bf16, tag="xls", name=f"xls{li}")
                src = dram[:].rearrange(
                    "(s p) (li n) -> li p s n", p=P, li=NLI)[li]
                nc.sync.dma_start(
                    xt[:, 0:W].rearrange("p (s n) -> p s n", s=ngrp), src)
                return xt

            def median_chunk(dram, nb, ngrp):
                """Suffix chain, then prefix chain with interleaved merge.
                Returns (cA3, cB3) views [P, nbt, KS]."""
                nbt = ngrp * nb
                W = nbt * 16
                nA = nbt - 1
                WA = nA * 16
                mk = mask[:, 0:W]
                t = pool.tile([P, WMAX], bf16, tag="t", name="t")
                tm = pool.tile([P, WMAX], bf16, tag="tm", name="tm")
                cA = pool.tile([P, NQ * NB_H * KS], bf16, tag="cA", name="cA")
                # cB gets a leading guard block: logical block b lives at
                # storage block b+1, so the final combine's in1 (logical
                # slot o-1, i.e. storage 32(b+1)+o-1) stays in-bounds at
                # o=0 and reads GUARD from the previous block's pad slot.
                cB = pool.tile([P, (NQ * NB_H + 1) * KS], bf16, tag="cB",
                               name="cB")
                cA3 = cA[:, 0:nbt * KS].rearrange("p (b k) -> p b k", k=KS)
                cB3 = cB[:, KS:(nbt + 1) * KS].rearrange(
                    "p (b k) -> p b k", k=KS)

                # ---- suffix chain: suf[l] compact k <-> logical 15-l+k
                xt = stream(dram, nb, ngrp, LEV)
                nc.vector.tensor_tensor_scan(
                    suf[0][:, 0:W][:, ::-1], mk, xt[:, 0:W][:, ::-1],
                    GUARD, op0=ADD, op1=MIN)
                for l in range(1, LEV):
                    xt = stream(dram, nb, ngrp, LEV + l)
                    nc.vector.tensor_tensor(
                        t[:, 0:W], suf[l - 1][:, 0:W], xt[:, 0:W], op=MAX)
                    nc.vector.tensor_tensor_scan(
                        suf[l][:, 0:W][:, ::-1], mk, t[:, 0:W][:, ::-1],
                        GUARD, op0=ADD, op1=MIN)
                # guard-init cmins (DMA, emitted after the stream DMAs so
                # the level streams win the queue; needed only at merge)
                nc.sync.dma_start(cA[:, 0:nbt * KS], GI[:, 0:nbt * KS])
                nc.sync.dma_start(cB[:, 0:(nbt + 1) * KS],
                                  GI[:, 0:(nbt + 1) * KS])
                # lay=16: pure suffix, o in [0,15]
                s153 = suf[15][:, 0:WA].rearrange("p (b k) -> p b k", k=16)
                nc.vector.tensor_tensor(
                    cA3[:, 0:nA, 0:16], cA3[:, 0:nA, 0:16], s153, op=MIN)

                # ---- prefix chain + interleaved merge layers
                xt = stream(dram, nb, ngrp, 0)
                nc.vector.tensor_tensor_scan(
                    prer[0][:, 0:W], mk, xt[:, 0:W], GUARD, op0=ADD, op1=MIN)
                for l in range(0, LEV):
                    pl = prer[l % 3]
                    if l < 15:
                        # merge layer lay = 15-l uses pre[l] & suf[14-l]
                        lay = 15 - l
                        nc.vector.tensor_tensor(
                            tm[:, 0:WA], suf[lay - 1][:, 0:WA],
                            pl[:, 16:W], op=MAX)
                        tm3 = tm[:, 0:WA].rearrange("p (b k) -> p b k", k=16)
                        if lay % 2 == 0:
                            dst = cA3[:, 0:nA, 16 - lay:32 - lay]
                        else:
                            dst = cB3[:, 0:nA, 15 - lay:31 - lay]
                        nc.vector.tensor_tensor(dst, dst, tm3, op=MIN)
                    else:
                        # lay=0: pure prefix pre[15], o in [16,30]
                        p153 = pl[:, 16:W].rearrange(
                            "p (b k) -> p b k", k=16)[:, :, 0:15]
                        nc.vector.tensor_tensor(
                            cA3[:, 0:nA, 16:31], cA3[:, 0:nA, 16:31],
                            p153, op=MIN)
                    if l < 15:
                        xt = stream(dram, nb, ngrp, l + 1)
                        nc.vector.tensor_tensor(
                            t[:, 0:W], pl[:, 0:W], xt[:, 0:W], op=MAX)
                        nc.vector.tensor_tensor_scan(
                            prer[(l + 1) % 3][:, 0:W], mk, t[:, 0:W],
                            GUARD, op0=ADD, op1=MIN)
                return cA3, cB3

            # ================= perc: one chunk, 4 strips
            pcomp = cpool.tile([P, 4 * 1056], bf16)
            pc4 = pcomp[:].rearrange("p (g l) -> p g l", g=4)
            cA3, cB3 = median_chunk(XPL, NB_P, NSTR)
            cA4 = cA3.rearrange("p (g b) k -> p g b k", g=NSTR)
            cB4 = cB3.rearrange("p (g b) k -> p g b k", g=NSTR)
            pc5 = pc4[:, :, 0:34 * K].rearrange("p g (b s) -> p g b s", s=K)
            nc.vector.tensor_tensor(
                pc5[:, :, :, 1:31], cA4[:, :, 0:34, 1:31],
                cB4[:, :, 0:34, 0:30], op=MIN)
            nc.vector.tensor_scalar_add(pc5[:, :, :, 0:1],
                                        cA4[:, :, 0:34, 0:1], 0.0)
            for cg in range(4):
                nc.sync.dma_start(PMR[cg * P:(cg + 1) * P, :],
                                  pc4[:, cg, 1024:1026])

            # perc medians transposed for the softmask (overlaps harm chain);
            # the PSUM->SBUF copy applies Square, so percT holds p^2
            percT = spool.tile([P, NQ * 512], bf16, tag="percT", name="percT")
            pT3 = percT[:].rearrange("p (q n) -> p q n", q=NQ)
            for qq in range(NQ):
                for cg in range(4):
                    ps = ppool.tile([P, P], bf16, tag="ps", name="ps")
                    nc.tensor.transpose(
                        ps[:], pc4[:, cg, qq * P:(qq + 1) * P], ident[:])
                    nc.scalar.activation(pT3[:, qq, cg * P:(cg + 1) * P],
                                         ps[:], AF.Square)

            xs = spool.tile([P, NQ * 512], bf16, tag="xs", name="xs")
            xs3 = xs[:].rearrange("p (q n) -> p q n", q=NQ)
            nc.sync.dma_start(
                xs3, XS[:].rearrange("(q p) n -> p q n", p=P))

            # ================= harm: one chunk, 8 row groups
            cA3, cB3 = median_chunk(XHL, NB_H, NQ)

            hc = spool.tile([P, NQ * 527], bf16, tag="hc", name="hc")
            hc5 = hc[:].rearrange("p (q l) -> p q l", q=NQ).rearrange(
                "p q (b s) -> p q b s", s=K)
            cA4 = cA3.rearrange("p (q b) k -> p q b k", q=NQ)
            cB4 = cB3.rearrange("p (q b) k -> p q b k", q=NQ)
            nc.vector.tensor_tensor(
                hc5[:, :, :, 1:31], cA4[:, :, 0:17, 1:31],
                cB4[:, :, 0:17, 0:30], op=MIN)
            nc.vector.tensor_scalar_add(hc5[:, :, :, 0:1],
                                        cA4[:, :, 0:17, 0:1], 0.0)

            # softmask in two halves, all on the DVE (no cross-engine hops
            # in the tail): h2 = hc*hc, den = h2+p2 (f32), r = 1/den,
            # oh = h2*r*S, op = S - oh
            hc4 = hc[:].rearrange("p (q l) -> p q l", q=NQ)
            HQ = NQ // 4
            for qtr in range(4):
                qs = slice(qtr * HQ, (qtr + 1) * HQ)
                h2 = spool.tile([P, HQ * 512], bf16, tag=f"h2{qtr % 2}",
                                name=f"h2{qtr % 2}")
                h23 = h2[:].rearrange("p (q n) -> p q n", q=HQ)
                nc.vector.tensor_tensor(
                    h23, hc4[:, qs, 0:512], hc4[:, qs, 0:512], op=MULT)
                den = spool.tile([P, HQ * 512], f32, tag=f"den{qtr % 2}",
                                 name=f"den{qtr % 2}")
                nc.vector.tensor_tensor(
                    den[:], h2[:], percT[:, qtr * HQ * 512:
                                         (qtr + 1) * HQ * 512], op=ADD)
                nc.vector.reciprocal_approx_fast(den[:], den[:])
                xsh = xs3[:, qs, :]
                nc.vector.tensor_tensor(h23, h23, den[:].rearrange(
                    "p (q n) -> p q n", q=HQ), op=MULT)
                nc.vector.tensor_tensor(h23, h23, xsh, op=MULT)
                nc.vector.tensor_tensor(pT3[:, qs, :], xsh, h23, op=SUB)
                oh_d = OH[:].rearrange("(h q p) n -> h p q n", p=P, q=HQ)[qtr]
                op_d = OP[:].rearrange("(h q p) n -> h p q n", p=P, q=HQ)[qtr]
                nc.sync.dma_start(oh_d, h23)
                nc.sync.dma_start(op_d, pT3[:, qs, :])

    nc.finalize()
    return nc


def _get_program():
    global _PROGRAM
    if _PROGRAM is None:
        _PROGRAM = _build_program()
    return _PROGRAM


def _level_idx(nb, limit):
    b = K * np.arange(nb)[None, :, None]
    k = np.arange(16)[None, None, :]
    l = np.arange(LEV)[:, None, None]
    pref = l + b + k
    sufx = (HALF - l) + b + k
    idx = np.concatenate([pref, sufx], axis=0)
    return np.minimum(idx, limit)


def _host_prep(S):
    import ml_dtypes

    bf = ml_dtypes.bfloat16
    ident = np.eye(P, dtype=np.float32).astype(bf)
    pidx = _level_idx(NB_P, 1085).reshape(-1)
    hidx = _level_idx(NB_H, 558).reshape(-1)
    in_maps = []
    for c in range(8):
        pl, h = c >> 1, c & 1
        b, ch = pl >> 1, pl & 1
        Sp = S[b, ch]
        xpl = np.zeros((512, 1086), np.float32)
        xpl[:, HALF:HALF + 1025] = Sp[:, 512 * h:512 * h + 512].T
        xplb = xpl[:, pidx].astype(bf)
        lo = 512 * h - HALF
        xhl = np.zeros((1024, 559), np.float32)
        s0, s1 = max(0, lo), min(1024, lo + 559)
        xhl[:, s0 - lo:s1 - lo] = Sp[0:1024, s0:s1]
        xhlb = xhl[:, hidx].astype(bf)
        xs = Sp[0:1024, 512 * h:512 * h + 512].astype(bf)
        gi = np.full((P, (NQ * NB_H + 1) * KS), GUARD, np.float32).astype(bf)
        in_maps.append({"XPL": xplb, "XHL": xhlb, "XS": xs, "ID": ident,
                        "GI": gi})
    return in_maps


def _median31_rows(rows):
    p = np.pad(rows, ((0, 0), (HALF, HALF)))
    win = np.lib.stride_tricks.sliding_window_view(p, K, axis=1)
    return np.median(win, axis=2).astype(np.float32)


def kernel(S):
    from concourse.bass_utils import run_bass_kernel_spmd

    S = np.asarray(S, np.float32)
    nc = _get_program()
    in_maps = _host_prep(S)
    res = run_bass_kernel_spmd(nc, in_maps, list(range(8)))

    out_h = np.empty_like(S)
    out_p = np.empty_like(S)
    perc_1024 = np.empty((2, 2, 1024), np.float32)
    for c in range(8):
        pl, h = c >> 1, c & 1
        b, ch = pl >> 1, pl & 1
        r = res.results[c]
        cols = slice(512 * h, 512 * h + 512)
        out_h[b, ch, 0:1024, cols] = np.asarray(r["OH"]).astype(np.float32)
        out_p[b, ch, 0:1024, cols] = np.asarray(r["OP"]).astype(np.float32)
        perc_1024[b, ch, cols] = np.asarray(r["PMR"])[:, 0].astype(np.float32)
    rows = S[:, :, 1024, :].reshape(4, 1024)
    harm_1024 = _median31_rows(rows).reshape(2, 2, 1024)
    h2 = harm_1024 * harm_1024
    p2 = perc_1024 * perc_1024
    rden = 1.0 / (h2 + p2)
    out_h[:, :, 1024, :] = S[:, :, 1024, :] * h2 * rden
    out_p[:, :, 1024, :] = S[:, :, 1024, :] * p2 * rden
    return out_h, out_p


# revision 19
# speedup vs baseline: 1.1965x; 1.1965x over previous
"""HPSS (harmonic/percussive source separation) Trainium2 kernel, v5.

Input S [2,2,1025,1024] f32. Per (b,c) plane: harm = median-31 along W
(zero-padded), perc = median-31 along H; softmask with power=2, margin=1;
returns (S*mask_h, S*mask_p).

Sharding: 8 cores = 4 planes x 2 W-halves. Each core computes perc medians
for its 512 columns over rows 0..1024 and harm medians + softmask outputs
for rows 0..1023 x its 512 columns. Row 1024 is finished on the host.

Exact median-31 via Gil-Werman prefix/suffix order statistics in bf16.
Each level l only ever feeds 16 slots per 31-block downstream, so levels
live in compact [*, nb, 16] tiles; every level's x-window is pre-gathered
on the host into contiguous DRAM streams (XPL/XHL) so all DVE tensor ops
run on contiguous bf16 operands (2x mode). The layer merge accumulates
min-over-layers into parity-split tiles (even layers -> cminA at slot o,
odd -> cminB at o-1; both 4B-aligned) over a contiguous block range;
cross-strip/cross-q garbage blocks land in output slots nothing reads.
Perc medians stay in SBUF; the PE transposes them for the softmask.

v5: one mega-chunk per orientation (perc: 4 column strips stacked on the
free axis, harm: all 8 row groups) to amortize the ~180ns/scan fixed cost
and DVE dispatch gaps. Fits in SBUF by running the suffix chain first and
interleaving merge layer 15-l right after prefix level l, so only 3
rotating pre tiles are live. reciprocal_approx_fast for the softmask.
"""
import sys

import numpy as np

sys.path.insert(0, "/opt/trn_rl_repo")

P = 128
K = 31
KS = 32
LEV = 16
GUARD = 2.0
NB_P = 35          # perc blocks per strip (covers padded-H 1085)
NB_H = 18          # harm blocks per row group (covers 558-col strip)
NSTR = 4           # perc strips (all 512 cols in one chunk)
NQ = 8             # harm row groups (all 1024 rows in one chunk)
HALF = 15
NLI = 2 * LEV      # 32 level streams

_PROGRAM = None


def _build_program():
    from contextlib import ExitStack

    import concourse.mybir as mybir
    import concourse.tile as tile
    from concourse import bacc

    f32 = mybir.dt.float32
    bf16 = mybir.dt.bfloat16
    MIN = mybir.AluOpType.min
    MAX = mybir.AluOpType.max
    ADD = mybir.AluOpType.add
    MULT = mybir.AluOpType.mult
    SUB = mybir.AluOpType.subtract

    from bass_rust import ActivationFunctionType as AF

    nc = bacc.Bacc("TRN2", target_bir_lowering=False, debug=True)
    XPL = nc.declare_dram_parameter("XPL", [512, NLI * NB_P * 16], bf16,
                                    isOutput=False)
    XHL = nc.declare_dram_parameter("XHL", [1024, NLI * NB_H * 16], bf16,
                                    isOutput=False)
    XS = nc.declare_dram_parameter("XS", [1024, 512], bf16, isOutput=False)
    ID = nc.declare_dram_parameter("ID", [P, P], bf16, isOutput=False)
    GI = nc.declare_dram_parameter("GI", [P, (NQ * NB_H + 1) * KS], bf16,
                                   isOutput=False)
    OH = nc.declare_dram_parameter("OH", [1024, 512], bf16, isOutput=True)
    OP = nc.declare_dram_parameter("OP", [1024, 512], bf16, isOutput=True)
    PMR = nc.declare_dram_parameter("PMR", [512, 2], bf16, isOutput=True)

    WMAX = NQ * NB_H * 16  # 2304 compact slots (harm); perc uses 2240

    with tile.TileContext(nc) as tc:
        with ExitStack() as ctx:
            cpool = ctx.enter_context(tc.tile_pool(name="const", bufs=1))
            inpool = ctx.enter_context(tc.tile_pool(name="in", bufs=3))
            pool = ctx.enter_context(tc.tile_pool(name="work", bufs=1))
            spool = ctx.enter_context(tc.tile_pool(name="soft", bufs=1))
            ppool = ctx.enter_context(tc.tile_pool(name="ps", bufs=2,
                                                   space="PSUM"))

            mask = cpool.tile([P, WMAX], bf16)
            nc.vector.memset(mask[:], 0.0)
            nc.vector.memset(
                mask[:].rearrange("p (b k) -> p b k", k=16)[:, :, 0:1], 1e30)
            ident = cpool.tile([P, P], bf16)
            nc.sync.dma_start(ident[:], ID[:])

            suf = [pool.tile([P, WMAX], bf16, tag=f"suf{l}", name=f"suf{l}")
                   for l in range(LEV)]
            prer = [pool.tile([P, WMAX], bf16, tag=f"prer{i}", name=f"prer{i}")
                    for i in range(3)]

            def stream(dram, nb, ngrp, li):
                W = ngrp * nb * 16
                xt = inpool.tile([P, WMAX], bf16, tag="xls", name=f"xls{li}")
                src = dram[:].rearrange(
                    "(s p) (li n) -> li p s n", p=P, li=NLI)[li]
                nc.sync.dma_start(
                    xt[:, 0:W].rearrange("p (s n) -> p s n", s=ngrp), src)
                return xt

            def median_chunk(dram, nb, ngrp):
                """Suffix chain, then prefix chain with interleaved merge.
                Returns (cA3, cB3) views [P, nbt, KS]."""
                nbt = ngrp * nb
                W = nbt * 16
                nA = nbt - 1
                WA = nA * 16
                mk = mask[:, 0:W]
                t = pool.tile([P, WMAX], bf16, tag="t", name="t")
                tm = pool.tile([P, WMAX], bf16, tag="tm", name="tm")
                cA = pool.tile([P, NQ * NB_H * KS], bf16, tag="cA", name="cA")
                # cB gets a leading guard block: logical block b lives at
                # storage block b+1, so the final combine's in1 (logical
                # slot o-1, i.e. storage 32(b+1)+o-1) stays in-bounds at
                # o=0 and reads GUARD from the previous block's pad slot.
                cB = pool.tile([P, (NQ * NB_H + 1) * KS], bf16, tag="cB",
                               name="cB")
                cA3 = cA[:, 0:nbt * KS].rearrange("p (b k) -> p b k", k=KS)
                cB3 = cB[:, KS:(nbt + 1) * KS].rearrange(
                    "p (b k) -> p b k", k=KS)

                # ---- suffix chain: suf[l] compact k <-> logical 15-l+k
                xt = stream(dram, nb, ngrp, LEV)
                nc.vector.tensor_tensor_scan(
                    suf[0][:, 0:W][:, ::-1], mk, xt[:, 0:W][:, ::-1],
                    GUARD, op0=ADD, op1=MIN)
                for l in range(1, LEV):
                    xt = stream(dram, nb, ngrp, LEV + l)
                    nc.vector.tensor_tensor(
                        t[:, 0:W], suf[l - 1][:, 0:W], xt[:, 0:W], op=MAX)
                    nc.vector.tensor_tensor_scan(
                        suf[l][:, 0:W][:, ::-1], mk, t[:, 0:W][:, ::-1],
                        GUARD, op0=ADD, op1=MIN)
                # guard-init cmins (DMA, emitted after the stream DMAs so
                # the level streams win the queue; needed only at merge)
                nc.sync.dma_start(cA[:, 0:nbt * KS], GI[:, 0:nbt * KS])
                nc.sync.dma_start(cB[:, 0:(nbt + 1) * KS],
                                  GI[:, 0:(nbt + 1) * KS])
                # lay=16: pure suffix, o in [0,15]
                s153 = suf[15][:, 0:WA].rearrange("p (b k) -> p b k", k=16)
                nc.vector.tensor_tensor(
                    cA3[:, 0:nA, 0:16], cA3[:, 0:nA, 0:16], s153, op=MIN)

                # ---- prefix chain + interleaved merge layers
                xt = stream(dram, nb, ngrp, 0)
                nc.vector.tensor_tensor_scan(
                    prer[0][:, 0:W], mk, xt[:, 0:W], GUARD, op0=ADD, op1=MIN)
                for l in range(0, LEV):
                    pl = prer[l % 3]
                    if l < 15:
                        # merge layer lay = 15-l uses pre[l] & suf[14-l]
                        lay = 15 - l
                        nc.vector.tensor_tensor(
                            tm[:, 0:WA], suf[lay - 1][:, 0:WA],
                            pl[:, 16:W], op=MAX)
                        tm3 = tm[:, 0:WA].rearrange("p (b k) -> p b k", k=16)
                        if lay % 2 == 0:
                            dst = cA3[:, 0:nA, 16 - lay:32 - lay]
                        else:
                            dst = cB3[:, 0:nA, 15 - lay:31 - lay]
                        nc.vector.tensor_tensor(dst, dst, tm3, op=MIN)
                    else:
                        # lay=0: pure prefix pre[15], o in [16,30]
                        p153 = pl[:, 16:W].rearrange(
                            "p (b k) -> p b k", k=16)[:, :, 0:15]
                        nc.vector.tensor_tensor(
                            cA3[:, 0:nA, 16:31], cA3[:, 0:nA, 16:31],
                            p153, op=MIN)
                    if l < 15:
                        xt = stream(dram, nb, ngrp, l + 1)
                        nc.vector.tensor_tensor(
                            t[:, 0:W], pl[:, 0:W], xt[:, 0:W], op=MAX)
                        nc.vector.tensor_tensor_scan(
                            prer[(l + 1) % 3][:, 0:W], mk, t[:, 0:W],
                            GUARD, op0=ADD, op1=MIN)
                return cA3, cB3

            # ================= perc: one chunk, 4 strips
            pcomp = cpool.tile([P, 4 * 1056], bf16)
            pc4 = pcomp[:].rearrange("p (g l) -> p g l", g=4)
            cA3, cB3 = median_chunk(XPL, NB_P, NSTR)
            cA4 = cA3.rearrange("p (g b) k -> p g b k", g=NSTR)
            cB4 = cB3.rearrange("p (g b) k -> p g b k", g=NSTR)
            pc5 = pc4[:, :, 0:34 * K].rearrange("p g (b s) -> p g b s", s=K)
            nc.vector.tensor_tensor(
                pc5[:, :, :, 1:31], cA4[:, :, 0:34, 1:31],
                cB4[:, :, 0:34, 0:30], op=MIN)
            nc.vector.tensor_scalar_add(pc5[:, :, :, 0:1],
                                        cA4[:, :, 0:34, 0:1], 0.0)
            for cg in range(4):
                nc.sync.dma_start(PMR[cg * P:(cg + 1) * P, :],
                                  pc4[:, cg, 1024:1026])

            # perc medians transposed for the softmask (overlaps harm chain);
            # the PSUM->SBUF copy applies Square, so percT holds p^2
            percT = spool.tile([P, NQ * 512], bf16, tag="percT", name="percT")
            pT3 = percT[:].rearrange("p (q n) -> p q n", q=NQ)
            for qq in range(NQ):
                for cg in range(4):
                    ps = ppool.tile([P, P], bf16, tag="ps", name="ps")
                    nc.tensor.transpose(
                        ps[:], pc4[:, cg, qq * P:(qq + 1) * P], ident[:])
                    nc.scalar.activation(pT3[:, qq, cg * P:(cg + 1) * P],
                                         ps[:], AF.Square)

            xs = spool.tile([P, NQ * 512], bf16, tag="xs", name="xs")
            xs3 = xs[:].rearrange("p (q n) -> p q n", q=NQ)
            nc.sync.dma_start(
                xs3, XS[:].rearrange("(q p) n -> p q n", p=P))

            # ================= harm: one chunk, 8 row groups
            cA3, cB3 = median_chunk(XHL, NB_H, NQ)

            hc = spool.tile([P, NQ * 527], bf16, tag="hc", name="hc")
            hc5 = hc[:].rearrange("p (q l) -> p q l", q=NQ).rearrange(
                "p q (b s) -> p q b s", s=K)
            cA4 = cA3.rearrange("p (q b) k -> p q b k", q=NQ)
            cB4 = cB3.rearrange("p (q b) k -> p q b k", q=NQ)
            nc.vector.tensor_tensor(
                hc5[:, :, :, 1:31], cA4[:, :, 0:17, 1:31],
                cB4[:, :, 0:17, 0:30], op=MIN)
            nc.vector.tensor_scalar_add(hc5[:, :, :, 0:1],
                                        cA4[:, :, 0:17, 0:1], 0.0)

            # softmask in two halves, all on the DVE (no cross-engine hops
            # in the tail): h2 = hc*hc, den = h2+p2 (f32), r = 1/den,
            # oh = h2*r*S, op = S - oh
            hc4 = hc[:].rearrange("p (q l) -> p q l", q=NQ)
            HQ = NQ // 2
            for half in range(2):
                qs = slice(half * HQ, (half + 1) * HQ)
                h2 = spool.tile([P, HQ * 512], bf16, tag=f"h2{half}",
                                name=f"h2{half}")
                h23 = h2[:].rearrange("p (q n) -> p q n", q=HQ)
                nc.vector.tensor_tensor(
                    h23, hc4[:, qs, 0:512], hc4[:, qs, 0:512], op=MULT)
                den = spool.tile([P, HQ * 512], f32, tag=f"den{half}",
                                 name=f"den{half}")
                nc.vector.tensor_tensor(
                    den[:], h2[:], percT[:, half * HQ * 512:
                                         (half + 1) * HQ * 512], op=ADD)
                nc.vector.reciprocal_approx_fast(den[:], den[:])
                xsh = xs3[:, qs, :]
                nc.vector.tensor_tensor(h23, h23, den[:].rearrange(
                    "p (q n) -> p q n", q=HQ), op=MULT)
                nc.vector.tensor_tensor(h23, h23, xsh, op=MULT)
                nc.vector.tensor_tensor(pT3[:, qs, :], xsh, h23, op=SUB)
                oh_d = OH[:].rearrange("(h q p) n -> h p q n", p=P, q=HQ)[half]
                op_d = OP[:].rearrange("(h q p) n -> h p q n", p=P, q=HQ)[half]
                nc.sync.dma_start(oh_d, h23)
                nc.sync.dma_start(op_d, pT3[:, qs, :])

    nc.finalize()
    return nc


def _get_program():
    global _PROGRAM
    if _PROGRAM is None:
        _PROGRAM = _build_program()
    return _PROGRAM


def _level_idx(nb, limit):
    b = K * np.arange(nb)[None, :, None]
    k = np.arange(16)[None, None, :]
    l = np.arange(LEV)[:, None, None]
    pref = l + b + k
    sufx = (HALF - l) + b + k
    idx = np.concatenate([pref, sufx], axis=0)
    return np.minimum(idx, limit)


def _host_prep(S):
    import ml_dtypes

    bf = ml_dtypes.bfloat16
    ident = np.eye(P, dtype=np.float32).astype(bf)
    pidx = _level_idx(NB_P, 1085).reshape(-1)
    hidx = _level_idx(NB_H, 558).reshape(-1)
    in_maps = []
    for c in range(8):
        pl, h = c >> 1, c & 1
        b, ch = pl >> 1, pl & 1
        Sp = S[b, ch]
        xpl = np.zeros((512, 1086), np.float32)
        xpl[:, HALF:HALF + 1025] = Sp[:, 512 * h:512 * h + 512].T
        xplb = xpl[:, pidx].astype(bf)
        lo = 512 * h - HALF
        xhl = np.zeros((1024, 559), np.float32)
        s0, s1 = max(0, lo), min(1024, lo + 559)
        xhl[:, s0 - lo:s1 - lo] = Sp[0:1024, s0:s1]
        xhlb = xhl[:, hidx].astype(bf)
        xs = Sp[0:1024, 512 * h:512 * h + 512].astype(bf)
        gi = np.full((P, (NQ * NB_H + 1) * KS), GUARD, np.float32).astype(bf)
        in_maps.append({"XPL": xplb, "XHL": xhlb, "XS": xs, "ID": ident,
                        "GI": gi})
    return in_maps


def _median31_rows(rows):
    p = np.pad(rows, ((0, 0), (HALF, HALF)))
    win = np.lib.stride_tricks.sliding_window_view(p, K, axis=1)
    return np.median(win, axis=2).astype(np.float32)


def kernel(S):
    from concourse.bass_utils import run_bass_kernel_spmd

    S = np.asarray(S, np.float32)
    nc = _get_program()
    in_maps = _host_prep(S)
    res = run_bass_kernel_spmd(nc, in_maps, list(range(8)))

    out_h = np.empty_like(S)
    out_p = np.empty_like(S)
    perc_1024 = np.empty((2, 2, 1024), np.float32)
    for c in range(8):
        pl, h = c >> 1, c & 1
        b, ch = pl >> 1, pl & 1
        r = res.results[c]
        cols = slice(512 * h, 512 * h + 512)
        out_h[b, ch, 0:1024, cols] = np.asarray(r["OH"]).astype(np.float32)
        out_p[b, ch, 0:1024, cols] = np.asarray(r["OP"]).astype(np.float32)
        perc_1024[b, ch, cols] = np.asarray(r["PMR"])[:, 0].astype(np.float32)
    rows = S[:, :, 1024, :].reshape(4, 1024)
    harm_1024 = _median31_rows(rows).reshape(2, 2, 1024)
    h2 = harm_1024 * harm_1024
    p2 = perc_1024 * perc_1024
    rden = 1.0 / (h2 + p2)
    out_h[:, :, 1024, :] = S[:, :, 1024, :] * h2 * rden
    out_p[:, :, 1024, :] = S[:, :, 1024, :] * p2 * rden
    return out_h, out_p
